# revision 1
# baseline (speedup 1.0000x reference)
"""Trainium2 Bass kernel for nn_Bogomol_89919435309160.

Data-parallel over batch: 16 samples -> 8 cores x 2 samples.
Each core runs the full per-sample pipeline:
  GroupNorm -> conv3x3+gelu -> patch-compress Linear -> LN(+pos) -> MHA
  -> dynamic filter synthesis -> per-sample 96x96x3x3 conv -> SE scaling.
"""

import os
import sys
import math
import numpy as np

for _p in ("/opt/trn_rl_repo", "/root/.axon_site/_ro/trn_rl_repo"):
    if os.path.isdir(_p) and _p not in sys.path:
        sys.path.insert(0, _p)

from contextlib import ExitStack  # noqa: E402
import concourse.bass as bass  # noqa: E402
import concourse.bacc as bacc  # noqa: E402
import concourse.tile as tile  # noqa: E402
from concourse import mybir  # noqa: E402
from concourse import bass_utils  # noqa: E402
import ml_dtypes  # noqa: E402

BF16 = ml_dtypes.bfloat16
F32 = np.float32
DBF = mybir.dt.bfloat16
DF32 = mybir.dt.float32
AF = mybir.ActivationFunctionType
ALU = mybir.AluOpType

# ---- problem dims (hardcoded per spec) ----
B, CIN, H, W = 16, 3, 224, 224
COUT, HID, NH = 96, 512, 4
PH = PW = 16
NHP, NWP = H // PH, W // PW   # 14, 14
SEQ = NHP * NWP               # 196
DH = HID // NH                # 128
NCORES = 8
BPC = B // NCORES             # 2
RS = W + 1                    # 225: feats row stride (shared pad col)
FROWS = H + 3                 # 227 rows in feats sbuf
NPIX = H * W                  # 50176
XP = 228                      # xn_pad row/col size
SH = 98                       # seq half-tile
ST = 2                        # seq tiles
CONV_ROWS = 14                # conv1 strip height (output rows)
NSTRIP = H // CONV_ROWS       # 16
EPS = 1e-5
ISQD = 1.0 / math.sqrt(DH)
AX = mybir.AxisListType


def apv(t, offset, dims):
    """Custom free-dim view of a tile AP; keeps partition dim."""
    a = t[:]
    return bass.AP(tensor=a.tensor, offset=a.offset + offset, ap=[a.ap[0]] + list(dims))


def build_program(phase=4):
    nc = bacc.Bacc("TRN2", target_bir_lowering=False, debug=False,
                   enable_asserts=False)

    def din(name, shape, dt):
        return nc.dram_tensor(name, shape, dt, kind="ExternalInput").ap()

    x_d = din("x", [BPC, CIN, H, W], DF32)
    compw_d = din("compw", [PH * PW, COUT, HID], DBF)      # [pypx, co, h]
    convw_d = din("convw", [27, COUT], DBF)                # [(dy,dx,c), co]
    convb_d = din("convb", [COUT, 1], DF32)
    gnw_d = din("gnw", [1, 2 * CIN], DF32)                 # g row + b row
    posb_d = din("posbT", [4, 128, SEQ], DF32)             # (pos+ln_b).T tiles
    lng_d = din("lngc", [128, 4], DF32)
    compb_d = din("compb", [1, HID], DBF)
    ones_d = din("ones", [1, HID], DBF)
    onescol_d = din("onescol", [128, 1], DF32)
    ident_f_d = din("identf", [128, 128], DF32)
    ident_b_d = din("identb", [128, 128], DBF)
    wq_d = din("wqT", [4, 128, HID], DBF)   # [j, jj, i*128+ii] = wq.T tiles
    wk_d = din("wkT", [4, 128, HID], DBF)
    wv_d = din("wvT", [4, 128, HID], DBF)
    wo_d = din("woT", [4, 128, HID], DBF)
    bq_d = din("bq", [1, HID], DBF)
    bk_d = din("bk", [1, HID], DBF)
    bv_d = din("bv", [1, HID], DBF)
    bo_d = din("bo", [128, 4], DF32)        # column per i-tile
    rw_d = din("rwT", [9, 128, 4, 128], DBF)  # [k, jj, j, co]
    rb_d = din("rb", [128, 9], DF32)          # re_b padded [co, k]
    bw_d = din("bwT", [128, 4, 9], DBF)       # be_w.T tiles
    bb_d = din("bb", [9, 1], DF32)
    pb_d = din("pbperm", [9, 864], DBF)      # patch_basis [k, off*96+ci]
    sw1_d = din("sw1T", [COUT, COUT // 4], DF32)   # se_w1.T
    sb1_d = din("sb1", [1, COUT // 4], DF32)
    sw2_d = din("sw2T", [COUT // 4, COUT], DF32)   # se_w2.T
    sb2_d = din("sb2", [COUT, 1], DF32)
    alpha_d = din("alphav", [1, 1], DF32)

    out_d = nc.dram_tensor("out", [BPC, COUT, H, W], DF32,
                           kind="ExternalOutput").ap()

    with tile.TileContext(nc) as tc, ExitStack() as ctx:
        pconst = ctx.enter_context(tc.tile_pool(name="const", bufs=1))
        pfeats = ctx.enter_context(tc.tile_pool(name="feats", bufs=1))
        pconv = ctx.enter_context(tc.tile_pool(name="conv", bufs=2))
        pcw = ctx.enter_context(tc.tile_pool(name="cw", bufs=4))
        pw = ctx.enter_context(tc.tile_pool(name="work", bufs=1))
        pwat = ctx.enter_context(tc.tile_pool(name="workat", bufs=1))
        prw = ctx.enter_context(tc.tile_pool(name="rwpool", bufs=4))
        pdyn = ctx.enter_context(tc.tile_pool(name="dyn", bufs=2))
        pdram = ctx.enter_context(tc.tile_pool(name="dramp", bufs=2, space="DRAM"))
        psA = ctx.enter_context(tc.tile_pool(name="psA", bufs=4, space="PSUM"))
        psB = ctx.enter_context(tc.tile_pool(name="psB", bufs=4, space="PSUM"))

        dma = nc.sync.dma_start

        _cseq = [0]

        def cload(dram_ap, shape, dt):
            _cseq[0] += 1
            t = pconst.tile(shape, dt, tag=f"c{_cseq[0]}")
            dma(t[:], dram_ap)
            return t

        convw_t = cload(convw_d, [27, COUT], DBF)
        convb_t = cload(convb_d, [COUT, 1], DF32)
        gnw_t = cload(gnw_d, [1, 2 * CIN], DF32)
        lngc_t = cload(lng_d, [128, 4], DF32)
        posbT_t = pconst.tile([128, 4, SEQ], DF32)
        for ht in range(4):
            dma(posbT_t[:, ht, :], posb_d[ht])
        compb_t = cload(compb_d, [1, HID], DBF)
        ones_t = cload(ones_d, [1, HID], DBF)
        onescol_t = cload(onescol_d, [128, 1], DF32)
        idf_t = cload(ident_f_d, [128, 128], DF32)
        idb_t = cload(ident_b_d, [128, 128], DBF)
        bo_t = cload(bo_d, [128, 4], DF32)
        rb_t = cload(rb_d, [128, 9], DF32)
        bb_t = cload(bb_d, [9, 1], DF32)
        pb_t = cload(pb_d, [9, 864], DBF)
        sw1_t = cload(sw1_d, [COUT, COUT // 4], DF32)
        sb1_t = cload(sb1_d, [1, COUT // 4], DF32)
        sw2_t = cload(sw2_d, [COUT // 4, COUT], DF32)
        sb2_t = cload(sb2_d, [COUT, 1], DF32)
        alpha_t = cload(alpha_d, [1, 1], DF32)
        eps1_t = pconst.tile([1, 1], DF32)
        nc.vector.memset(eps1_t[:], EPS)
        epsS_t = pconst.tile([SH, 1], DF32)
        nc.vector.memset(epsS_t[:], EPS)
        zrow = pconst.tile([1, XP], DBF)
        nc.vector.memset(zrow[:], 0.0)

        # feats buffer, allocated once; zero the pad cells once.
        feats_t = pfeats.tile([COUT, FROWS, RS], DBF)
        nc.vector.memset(feats_t[:, 0, :], 0.0)       # top pad row
        nc.vector.memset(feats_t[:, H + 1, :], 0.0)   # bottom pad row
        nc.vector.memset(feats_t[:, :, 0:1], 0.0)     # left pads (shared right pads)

        for smp in range(BPC):
            # ====================== GroupNorm ======================
            xt = pw.tile([112, CIN, 448], DF32, tag="scr5")
            for c in range(CIN):
                dma(xt[:, c, :], x_d[smp, c].rearrange("(a b) w -> a (b w)", b=2))
            stats = pw.tile([112, CIN, 6], DF32, tag="gnstats")
            for c in range(CIN):
                nc.vector.bn_stats(stats[:, c, :], xt[:, c, :])
            mv = pw.tile([112, 2], DF32, tag="gnmv")
            nc.vector.bn_aggr(mv[:], stats[:])
            m2 = pw.tile([112, 1], DF32, tag="gnm2")
            nc.vector.tensor_mul(m2[:], mv[:, 0:1], mv[:, 0:1])
            pg = psB.tile([1, 3], DF32, tag="ps")
            nc.tensor.matmul(pg[:, 0:2], onescol_t[:112, :], mv[:],
                             start=True, stop=True)
            nc.tensor.matmul(pg[:, 2:3], onescol_t[:112, :], m2[:],
                             start=True, stop=True)
            gs = pw.tile([1, 8], DF32, tag="gnsc")
            # cols of gs: 0 mu, 1 var, 2 sd, 3 rstd, 4 scratch
            nc.vector.tensor_scalar_mul(gs[:, 0:3], pg[:], 1.0 / 112.0)
            nc.vector.tensor_mul(gs[:, 4:5], gs[:, 0:1], gs[:, 0:1])
            nc.vector.tensor_add(gs[:, 1:2], gs[:, 1:2], gs[:, 2:3])
            nc.vector.tensor_sub(gs[:, 1:2], gs[:, 1:2], gs[:, 4:5])
            nc.scalar.activation(gs[:, 2:3], gs[:, 1:2], AF.Sqrt,
                                 bias=eps1_t[:], scale=1.0)
            nc.vector.reciprocal(gs[:, 3:4], gs[:, 2:3])
            nc.vector.tensor_scalar_mul(gs[:, 4:5], gs[:, 0:1], -1.0)
            ab = pw.tile([1, 2 * CIN], DF32, tag="gnab")
            nc.vector.tensor_scalar(ab[:, 0:CIN], gnw_t[:, 0:CIN],
                                    gs[:, 3:4], None, op0=ALU.mult)
            nc.vector.scalar_tensor_tensor(ab[:, CIN:], ab[:, 0:CIN],
                                           gs[:, 4:5], gnw_t[:, CIN:],
                                           op0=ALU.mult, op1=ALU.add)
            abr = pw.tile([112, 2 * CIN], DF32, tag="gnabr")
            nc.gpsimd.partition_broadcast(abr[:], ab[:])
            xnt = pw.tile([112, CIN, 448], DBF, tag="xnt")
            for c in range(CIN):
                nc.vector.tensor_scalar(xnt[:, c, :], xt[:, c, :],
                                        abr[:, c:c + 1], abr[:, CIN + c:CIN + c + 1],
                                        op0=ALU.mult, op1=ALU.add)
            xn_pad = pdram.tile([CIN, XP, XP], DBF)
            for c in range(CIN):
                dma(xn_pad[c, 1:225, 1:225].rearrange("(p r) f -> p r f", r=2),
                    xnt[:, c, :].rearrange("p (r f) -> p r f", r=2))
                dma(xn_pad[c, 0:1, :], zrow[:, 0:XP])
                dma(xn_pad[c, 225:226, :], zrow[:, 0:XP])
                dma(xn_pad[c:c + 1, 1:225, 0:1],
                    zrow[:, 0:224].rearrange("p (f t) -> p f t", t=1))
                dma(xn_pad[c:c + 1, 1:225, 225:226],
                    zrow[:, 0:224].rearrange("p (f t) -> p f t", t=1))
                dma(xn_pad[c:c + 1, 1:225, 226:227],
                    zrow[:, 0:224].rearrange("p (f t) -> p f t", t=1))

            # ====================== conv1 + gelu ======================
            fsum_cols = pw.tile([COUT, NSTRIP * (CONV_ROWS // 2)], DF32, tag="fsc")
            for s in range(NSTRIP):
                y0 = s * CONV_ROWS
                xr = pconv.tile([27, CONV_ROWS, RS], DBF, tag="xr")
                for off in range(9):
                    dy, dx = off // 3, off % 3
                    dma(xr[3 * off:3 * off + 3, :, :],
                        xn_pad[:, y0 + dy:y0 + dy + CONV_ROWS, dx:dx + RS])
                for r in range(CONV_ROWS // 2):
                    ps = psA.tile([COUT, 448], DF32, tag="ps")
                    nc.tensor.matmul(ps[:], convw_t[:], xr[:, 2 * r:2 * r + 2, 0:224],
                                     start=True, stop=True)
                    ti = s * (CONV_ROWS // 2) + r
                    nc.scalar.activation(
                        feats_t[:, y0 + 2 * r + 1:y0 + 2 * r + 3, 1:225],
                        ps[:].rearrange("p (r f) -> p r f", r=2),
                        AF.Gelu, bias=convb_t[:], scale=1.0,
                        accum_out=fsum_cols[:, ti:ti + 1])
            fsum = pw.tile([COUT, 1], DF32, tag="fsum")
            nc.vector.reduce_sum(fsum[:], fsum_cols[:], axis=AX.X)

            # edge sums for mean(sc)
            ed = pw.tile([COUT, 4], DF32, tag="ed")  # R0 R223 C0 C223
            nc.vector.reduce_sum(ed[:, 0:1], feats_t[:, 1, 1:225], axis=AX.X)
            nc.vector.reduce_sum(ed[:, 1:2], feats_t[:, 224, 1:225], axis=AX.X)
            nc.vector.reduce_sum(ed[:, 2:3], feats_t[:, 1:225, 1], axis=AX.X)
            nc.vector.reduce_sum(ed[:, 3:4], feats_t[:, 1:225, 224], axis=AX.X)
            corn = pw.tile([COUT, 2, 2], DF32, tag="corn")
            nc.vector.tensor_copy(corn[:, 0, :],
                                  apv(feats_t, 1 * RS + 1, [[223, 2]]))
            nc.vector.tensor_copy(corn[:, 1, :],
                                  apv(feats_t, 224 * RS + 1, [[223, 2]]))
            wsum = pw.tile([COUT, 9], DF32, tag="wsum")
            tmp1 = pw.tile([COUT, 1], DF32, tag="wtmp")
            for off in range(9):
                dy, dx = off // 3, off % 3
                av = ed[:, 1:2] if dy == 0 else (ed[:, 0:1] if dy == 2 else None)
                bv = ed[:, 3:4] if dx == 0 else (ed[:, 2:3] if dx == 2 else None)
                dst = wsum[:, off:off + 1]
                if av is None and bv is None:
                    nc.vector.tensor_copy(dst, fsum[:])
                elif av is None:
                    nc.vector.tensor_sub(dst, fsum[:], bv)
                elif bv is None:
                    nc.vector.tensor_sub(dst, fsum[:], av)
                else:
                    iy = 1 if dy == 0 else 0
                    ix = 1 if dx == 0 else 0
                    nc.vector.tensor_sub(tmp1[:], av, corn[:, iy, ix:ix + 1])
                    nc.vector.tensor_sub(dst, fsum[:], bv)
                    nc.vector.tensor_sub(dst, dst, tmp1[:])

            if phase <= 1:
                stg = pdyn.tile([COUT, 448], DF32, tag="so")
                nc.vector.tensor_copy(stg[:].rearrange("p (r f) -> p r f", r=2),
                                      feats_t[:, 1:3, 1:225])
                dma(out_d[smp, :, 0:2, :],
                    stg[:].rearrange("p (r f) -> p r f", r=2))
                continue

            # ====================== patch compress ======================
            # feature-major: e[ht*128+hh, s], accumulate 256 (py,px) matmuls
            pes = [psA.tile([128, SEQ], DF32, tag="ps", name=f"pe{_i}")
                   for _i in range(4)]
            for py in range(PH):
                for px in range(PW):
                    k = py * PW + px
                    cw = pcw.tile([COUT, HID], DBF, tag="cw")
                    dma(cw[:], compw_d[k])
                    mov = apv(feats_t, (py + 1) * RS + 1 + px,
                              [[16 * RS, 14], [16, 14]])
                    for ht in range(4):
                        nc.tensor.matmul(pes[ht][:],
                                         cw[:, ht * 128:(ht + 1) * 128], mov,
                                         start=(k == 0), stop=False)
            e_t = pw.tile([128, 4, SEQ], DF32, tag="e")
            for ht in range(4):
                nc.tensor.matmul(pes[ht][:], compb_t[:, ht * 128:(ht + 1) * 128],
                                 ones_t[:, 0:SEQ], start=False, stop=True)
                nc.scalar.activation(e_t[:, ht, :], pes[ht][:], AF.Gelu, scale=1.0)

            # ====================== LN + pos (feature-major) ======================
            psl1 = psB.tile([1, SEQ], DF32, tag="ps")
            psl2 = psB.tile([1, SEQ], DF32, tag="ps")
            esq = pw.tile([128, SEQ], DF32, tag="esq")
            for ht in range(4):
                nc.tensor.matmul(psl1[:], onescol_t[:, 0:1], e_t[:, ht, :],
                                 start=(ht == 0), stop=(ht == 3))
            for ht in range(4):
                nc.vector.tensor_mul(esq[:], e_t[:, ht, :], e_t[:, ht, :])
                nc.tensor.matmul(psl2[:], onescol_t[:, 0:1], esq[:],
                                 start=(ht == 0), stop=(ht == 3))
            lnr = pw.tile([1, 4, SEQ], DF32, tag="lnrow")
            nc.vector.tensor_scalar_mul(lnr[:, 0, :], psl1[:], 1.0 / HID)
            nc.vector.tensor_scalar_mul(lnr[:, 1, :], psl2[:], 1.0 / HID)
            nc.vector.tensor_mul(lnr[:, 2, :], lnr[:, 0, :], lnr[:, 0, :])
            nc.vector.tensor_sub(lnr[:, 1, :], lnr[:, 1, :], lnr[:, 2, :])
            nc.scalar.activation(lnr[:, 2, :], lnr[:, 1, :], AF.Sqrt,
                                 bias=eps1_t[:], scale=1.0)
            nc.vector.reciprocal(lnr[:, 3, :], lnr[:, 2, :])
            nc.vector.tensor_copy(lnr[:, 1, :], lnr[:, 3, :])
            mrep = pw.tile([128, 2, SEQ], DF32, tag="mrep")
            nc.gpsimd.partition_broadcast(mrep[:], lnr[:, 0:2, :])
            h_t = pw.tile([128, 4, SEQ], DBF, tag="h")
            for ht in range(4):
                tn = pw.tile([128, SEQ], DF32, tag="lntmp")
                nc.vector.tensor_sub(tn[:], e_t[:, ht, :], mrep[:, 0, :])
                nc.vector.tensor_mul(tn[:], tn[:], mrep[:, 1, :])
                nc.vector.scalar_tensor_tensor(h_t[:, ht, :], tn[:],
                                               lngc_t[:, ht:ht + 1],
                                               posbT_t[:, ht, :],
                                               op0=ALU.mult, op1=ALU.add)

            if phase <= 2:
                stg = pdyn.tile([COUT, 448], DF32, tag="so")
                nc.vector.tensor_copy(stg[:, 0:196], h_t[0:96, 0, :])
                nc.vector.memset(stg[:, 196:448], 0.0)
                dma(out_d[smp, :, 0:2, :],
                    stg[:].rearrange("p (r f) -> p r f", r=2))
                continue

            # ====================== attention ======================
            bqkv_t = pwat.tile([1, 3, HID], DBF, tag="bqkv")
            dma(bqkv_t[:, 0, :], bq_d)
            dma(bqkv_t[:, 1, :], bk_d)
            dma(bqkv_t[:, 2, :], bv_d)

            qT_t = pw.tile([128, 4, SEQ], DBF, tag="qat")
            kT_t = pw.tile([128, 4, SEQ], DBF, tag="kT")
            for dst_t, wd, bi in ((qT_t, wq_d, 0), (kT_t, wk_d, 1)):
                for i in range(4):
                    wqi = prw.tile([128, 4, 128], DBF, tag="rw")
                    for j in range(4):
                        dma(wqi[:, j, :], wd[j, :, i * 128:(i + 1) * 128])
                    pq = psB.tile([128, SEQ], DF32, tag="ps")
                    for j in range(4):
                        nc.tensor.matmul(pq[:], wqi[:, j, :],
                                         h_t[:, j, :], start=(j == 0), stop=False)
                    nc.tensor.matmul(pq[:], bqkv_t[:, bi, i * 128:(i + 1) * 128],
                                     ones_t[:, 0:SEQ], start=False, stop=True)
                    nc.vector.tensor_copy(dst_t[:, i, :], pq[:])
            v_t = pw.tile([SH, ST, HID], DBF, tag="v")
            for st in range(ST):
                pv = psB.tile([SH, HID], DF32, tag="ps")
                for j in range(4):
                    wvj = prw.tile([128, HID], DBF, tag="rw")
                    dma(wvj[:], wv_d[j])
                    nc.tensor.matmul(pv[:], h_t[:, j, st * SH:(st + 1) * SH],
                                     wvj[:], start=(j == 0), stop=False)
                nc.tensor.matmul(pv[:], ones_t[:, 0:SH], bqkv_t[:, 2, :],
                                 start=False, stop=True)
                nc.vector.tensor_copy(v_t[:, st, :], pv[:])

            o_t = pw.tile([SH, ST, HID], DBF, tag="o")
            for i in range(NH):
                for st in range(ST):
                    psS = psB.tile([SH, SEQ], DF32, tag="ps")
                    nc.tensor.matmul(psS[:], qT_t[:, i, st * SH:(st + 1) * SH],
                                     kT_t[:, i, :], start=True, stop=True)
                    mx = pw.tile([SH, 4], DF32, tag="mx")
                    nc.vector.reduce_max(mx[:, 0:1], psS[:], axis=AX.X)
                    nc.vector.tensor_scalar_mul(mx[:, 1:2], mx[:, 0:1], -ISQD)
                    P_t = pw.tile([SH, SEQ], DBF, tag="P")
                    nc.scalar.activation(P_t[:], psS[:], AF.Exp,
                                         bias=mx[:, 1:2], scale=ISQD)
                    nc.vector.reduce_sum(mx[:, 2:3], P_t[:], axis=AX.X)
                    nc.vector.reciprocal(mx[:, 3:4], mx[:, 2:3])
                    PT_t = pw.tile([SH, ST, SH], DBF, tag="PT")
                    for kt in range(ST):
                        pp = psB.tile([SH, SH], DBF, tag="ps")
                        nc.tensor.transpose(pp[:], P_t[:, kt * SH:(kt + 1) * SH],
                                            idb_t[:SH, :SH])
                        nc.vector.tensor_copy(PT_t[:, kt, :], pp[:])
                    po = psB.tile([SH, DH], DF32, tag="ps")
                    for kt in range(ST):
                        nc.tensor.matmul(po[:], PT_t[:, kt, :],
                                         v_t[:, kt, i * 128:(i + 1) * 128],
                                         start=(kt == 0), stop=(kt == ST - 1))
                    nc.vector.tensor_scalar_mul(o_t[:, st, i * 128:(i + 1) * 128],
                                                po[:], mx[:, 3:4])

            oT_t = pw.tile([128, 4, SEQ], DBF, tag="jT")
            for j in range(4):
                for st in range(ST):
                    pt = psB.tile([128, SH], DBF, tag="ps")
                    nc.tensor.transpose(pt[:], o_t[:, st, j * 128:(j + 1) * 128],
                                        idb_t[:SH, :SH])
                    nc.vector.tensor_copy(oT_t[:, j, st * SH:(st + 1) * SH], pt[:])
            attT_t = pw.tile([128, 4, SEQ], DBF, tag="qat")
            for i in range(4):
                woi = prw.tile([128, 4, 128], DBF, tag="rw")
                for j in range(4):
                    dma(woi[:, j, :], wo_d[j, :, i * 128:(i + 1) * 128])
                pa = psB.tile([128, SEQ], DF32, tag="ps")
                for j in range(4):
                    nc.tensor.matmul(pa[:], woi[:, j, :],
                                     oT_t[:, j, :], start=(j == 0), stop=(j == 3))
                nc.scalar.activation(attT_t[:, i, :], pa[:], AF.Gelu,
                                     bias=bo_t[:, i:i + 1], scale=1.0)

            # rep / votes / filt
            rep_t = pw.tile([128, 9], DF32, tag="rep")
            for k in range(9):
                rw_t = prw.tile([128, 4, 128], DBF, tag="rw")
                dma(rw_t[:], rw_d[k])
                pr = psB.tile([128, SEQ], DF32, tag="ps")
                for j in range(4):
                    nc.tensor.matmul(pr[:], rw_t[:, j, :], attT_t[:, j, :],
                                     start=(j == 0), stop=(j == 3))
                rg = pw.tile([128, SEQ], DBF, tag="rg")
                nc.scalar.activation(rg[:], pr[:], AF.Gelu,
                                     bias=rb_t[:, k:k + 1], scale=1.0)
                nc.vector.reduce_sum(rep_t[:, k:k + 1], rg[:], axis=AX.X)
            bw_t = pwat.tile([128, 4, 9], DBF, tag="bw")
            dma(bw_t[:], bw_d)
            pvv = psB.tile([9, SEQ], DF32, tag="ps")
            for j in range(4):
                nc.tensor.matmul(pvv[:], bw_t[:, j, :], attT_t[:, j, :],
                                 start=(j == 0), stop=(j == 3))
            votes_t = pw.tile([9, SEQ], DF32, tag="votes")
            nc.scalar.activation(votes_t[:], pvv[:], AF.Tanh, bias=bb_t[:], scale=1.0)
            vsum = pw.tile([9, 2], DF32, tag="vsum")
            nc.vector.reduce_sum(vsum[:, 0:1], votes_t[:], axis=AX.X)
            nc.vector.tensor_scalar_mul(vsum[:, 1:2], vsum[:, 0:1],
                                        1.0 / (SEQ * SEQ))
            pgt = psB.tile([9, 128], DF32, tag="ps")
            nc.tensor.transpose(pgt[:], rep_t[:], idf_t[:, :])
            gT_t = pw.tile([9, 128], DBF, tag="gT")
            nc.vector.tensor_scalar_mul(gT_t[:], pgt[:], vsum[:, 1:2])
            pf1 = psB.tile([COUT, 512], DF32, tag="ps")
            pf2 = psB.tile([COUT, 352], DF32, tag="ps")
            nc.tensor.matmul(pf1[:], gT_t[:, 0:COUT], pb_t[:, 0:512],
                             start=True, stop=True)
            nc.tensor.matmul(pf2[:], gT_t[:, 0:COUT], pb_t[:, 512:864],
                             start=True, stop=True)
            fperm = pw.tile([COUT, 864], DBF, tag="scr5")
            nc.vector.tensor_copy(fperm[:, 0:512], pf1[:])
            nc.vector.tensor_copy(fperm[:, 512:864], pf2[:])
            filtT_t = pw.tile([COUT, 9, COUT], DBF, tag="filtT")
            for off in range(9):
                pft = psB.tile([COUT, COUT], DBF, tag="ps")
                nc.tensor.transpose(pft[:], fperm[:, off * COUT:(off + 1) * COUT],
                                    idb_t[:COUT, :COUT])
                nc.vector.tensor_copy(filtT_t[:, off, :], pft[:])

            # ---------------- SE scalars ----------------
            wsum_b = pw.tile([COUT, 9], DBF, tag="wsumb")
            nc.vector.tensor_copy(wsum_b[:], wsum[:])
            pms = psB.tile([COUT, 1], DF32, tag="ps")
            for off in range(9):
                nc.tensor.matmul(pms[:], filtT_t[:, off, :], wsum_b[:, off:off + 1],
                                 start=(off == 0), stop=(off == 8))
            msc = pw.tile([COUT, 4], DF32, tag="msc")
            nc.vector.tensor_scalar_mul(msc[:, 0:1], pms[:], 1.0 / NPIX)
            abr2 = pw.tile([COUT, 2], DF32, tag="alco")
            nc.gpsimd.partition_broadcast(abr2[:, 0:1], alpha_t[:])
            nc.vector.tensor_scalar_mul(abr2[:, 1:2], abr2[:, 0:1], 1.0 / NPIX)
            nc.vector.scalar_tensor_tensor(msc[:, 1:2], fsum[:], abr2[:, 1:2],
                                           msc[:, 0:1], op0=ALU.mult, op1=ALU.add)
            ps1 = psB.tile([1, COUT // 4], DF32, tag="ps")
            nc.tensor.matmul(ps1[:], msc[:, 1:2], sw1_t[:], start=True, stop=True)
            se1 = pw.tile([1, COUT // 4], DF32, tag="se1")
            nc.vector.tensor_add(se1[:], ps1[:], sb1_t[:])
            nc.vector.tensor_scalar_max(se1[:], se1[:], 0.0)
            ps1t = psB.tile([COUT // 4, 1], DF32, tag="ps")
            nc.tensor.transpose(ps1t[:], se1[:], idf_t[:1, :1])
            se1c = pw.tile([COUT // 4, 1], DF32, tag="se1c")
            nc.vector.tensor_copy(se1c[:], ps1t[:])
            ps2 = psB.tile([COUT, 1], DF32, tag="ps")
            nc.tensor.matmul(ps2[:], sw2_t[:], se1c[:], start=True, stop=True)
            ssc = pw.tile([COUT, 4], DF32, tag="sse")
            nc.vector.tensor_add(ssc[:, 0:1], ps2[:], sb2_t[:])
            nc.scalar.activation(ssc[:, 1:2], ssc[:, 0:1], AF.Tanh,
                                 bias=0.0, scale=0.5)
            nc.vector.tensor_scalar(ssc[:, 2:3], ssc[:, 1:2], 0.5, 0.5,
                                    op0=ALU.mult, op1=ALU.add)
            nc.vector.tensor_mul(ssc[:, 3:4], ssc[:, 2:3], abr2[:, 0:1])

            if phase <= 3:
                stg = pdyn.tile([COUT, 448], DF32, tag="so")
                nc.vector.tensor_copy(stg[:, 0:96], filtT_t[:, 0, :])
                nc.vector.tensor_scalar(stg[:, 96:97], ssc[:, 2:3], 1.0, None,
                                        op0=ALU.mult)
                nc.vector.memset(stg[:, 97:448], 0.0)
                dma(out_d[smp, :, 0:2, :],
                    stg[:].rearrange("p (r f) -> p r f", r=2))
                continue

            # ====================== dynamic conv + out ======================
            for t in range(H // 2):
                y0 = 2 * t
                pd = psA.tile([COUT, 448], DF32, tag="ps")
                for off in range(9):
                    dy, dx = off // 3, off % 3
                    nc.tensor.matmul(pd[:], filtT_t[:, off, :],
                                     apv(feats_t, (y0 + dy) * RS + dx,
                                         [[RS, 2], [1, 224]]),
                                     start=(off == 0), stop=(off == 8))
                ft = pdyn.tile([COUT, 448], DF32, tag="ftmp")
                nc.scalar.activation(ft[:].rearrange("p (r f) -> p r f", r=2),
                                     feats_t[:, y0 + 1:y0 + 3, 1:225],
                                     AF.Copy, bias=0.0, scale=ssc[:, 3:4])
                so = pdyn.tile([COUT, 448], DF32, tag="so")
                nc.vector.scalar_tensor_tensor(so[:], pd[:], ssc[:, 2:3], ft[:],
                                               op0=ALU.mult, op1=ALU.add)
                dma(out_d[smp, :, y0:y0 + 2, :],
                    so[:].rearrange("p (r f) -> p r f", r=2))

            if os.environ.get("KERN_DEBUG_POOLS"):
                for p in (pconst, pfeats, pconv, pcw, pw, pwat, prw, pdyn):
                    print("POOL", p.name,
                          round(p.current_size() / 128 / 1024, 1), "KB/part")
                for p in (psA, psB):
                    print("PSUM POOL", p.name, p.current_size() / 128, "B/part")
                raise SystemExit(0)

    nc.compile()
    return nc


# ---------------- host-side prep ----------------

def _prep_shared(inputs):
    f = lambda a: np.asarray(a, dtype=F32)
    comp_w = f(inputs["comp_w"])      # [512, 24576]
    cw = comp_w.reshape(HID, COUT, PH * PW).transpose(2, 1, 0)
    compw = np.ascontiguousarray(cw).astype(BF16)
    conv_w = f(inputs["conv_w"])      # [96, 3, 3, 3]
    convw = np.ascontiguousarray(
        conv_w.transpose(2, 3, 1, 0).reshape(27, COUT)).astype(BF16)
    convb = np.ascontiguousarray(f(inputs["conv_b"]).reshape(COUT, 1))
    gnw = np.concatenate([f(inputs["gn_g"]), f(inputs["gn_b"])]).reshape(1, 6)
    ii, jj = np.meshgrid(np.arange(NHP, dtype=F32) / NHP,
                         np.arange(NWP, dtype=F32) / NWP, indexing="ij")
    coords = np.stack([ii, jj], -1).reshape(SEQ, 2)
    pos = np.tanh(coords @ f(inputs["pos_w"]).T + f(inputs["pos_b"]))
    posb = np.ascontiguousarray(
        (pos + f(inputs["ln_b"])[None, :]).T.reshape(4, 128, SEQ)).astype(F32)
    lng = np.ascontiguousarray(f(inputs["ln_g"]).reshape(4, 128).T)
    compb = f(inputs["comp_b"]).reshape(1, HID).astype(BF16)
    ones_r = np.ones((1, HID), dtype=BF16)
    onescol = np.ones((128, 1), dtype=F32)
    identf = np.eye(128, dtype=F32)
    identb = np.eye(128, dtype=BF16)

    def wtiles(w):  # [512,512] -> [4, 128, 512] tiles of w.T
        return np.ascontiguousarray(f(w).T.reshape(4, 128, HID)).astype(BF16)

    wqT, wkT, wvT, woT = (wtiles(inputs[k]) for k in ("wq", "wk", "wv", "wo"))
    bq = f(inputs["bq"]).reshape(1, HID).astype(BF16)
    bk = f(inputs["bk"]).reshape(1, HID).astype(BF16)
    bv = f(inputs["bv"]).reshape(1, HID).astype(BF16)
    bo = np.ascontiguousarray(f(inputs["bo"]).reshape(4, 128).T).astype(F32)
    re_w = f(inputs["re_w"])          # [864, 512], row co*9+k
    A = np.zeros((9, 128, HID), dtype=F32)
    A[:, :COUT, :] = re_w.reshape(COUT, 9, HID).transpose(1, 0, 2)
    # [k, j, jj, co] -> [k, jj, j, co]
    rwT = np.ascontiguousarray(
        A.transpose(0, 2, 1).reshape(9, 4, 128, 128).transpose(0, 2, 1, 3)
    ).astype(BF16)
    rb = np.zeros((128, 9), dtype=F32)
    rb[:COUT, :] = f(inputs["re_b"]).reshape(COUT, 9)
    be_w = f(inputs["be_w"])          # [9, 512]
    bwT = np.ascontiguousarray(
        be_w.T.reshape(4, 128, 9).transpose(1, 0, 2)).astype(BF16)
    bb = np.ascontiguousarray(f(inputs["be_b"]).reshape(9, 1))
    pb = f(inputs["patch_basis"])     # [9, 864] d = ci*9+off
    pbperm = np.ascontiguousarray(
        pb.reshape(9, COUT, 9).transpose(0, 2, 1).reshape(9, 864)).astype(BF16)
    sw1T = np.ascontiguousarray(f(inputs["se_w1"]).T)   # [96, 24]
    sb1 = np.ascontiguousarray(f(inputs["se_b1"]).reshape(1, COUT // 4))
    sw2T = np.ascontiguousarray(f(inputs["se_w2"]).T)   # [24, 96]
    sb2 = np.ascontiguousarray(f(inputs["se_b2"]).reshape(COUT, 1))
    alphav = f(inputs["alpha"]).reshape(1, 1)
    return dict(compw=compw, convw=convw, convb=convb, gnw=gnw, posbT=posb,
                lngc=lng, compb=compb, ones=ones_r, onescol=onescol,
                identf=identf, identb=identb, wqT=wqT, wkT=wkT, wvT=wvT,
                woT=woT, bq=bq, bk=bk, bv=bv, bo=bo, rwT=rwT, rb=rb,
                bwT=bwT, bb=bb, pbperm=pbperm, sw1T=sw1T, sb1=sb1,
                sw2T=sw2T, sb2=sb2, alphav=alphav)


_CACHE = {}


def _get_program():
    if "nc" not in _CACHE:
        _CACHE["nc"] = build_program(int(os.environ.get("KERN_PHASE", "4")))
    return _CACHE["nc"]


def make_in_maps(inputs):
    shared = _prep_shared(inputs)
    x = np.asarray(inputs["x"], dtype=F32)
    in_maps = []
    for c in range(NCORES):
        m = dict(shared)
        m["x"] = np.ascontiguousarray(x[c * BPC:(c + 1) * BPC])
        in_maps.append(m)
    return in_maps


def run(inputs, trace=False, **kw):
    nc = _get_program()
    in_maps = make_in_maps(inputs)
    res = bass_utils.run_bass_kernel_spmd(nc, in_maps, list(range(NCORES)),
                                          trace=trace, **kw)
    out = np.concatenate([np.asarray(r["out"]) for r in res.results], axis=0)
    return out, res


def kernel(**inputs):
    return run(inputs)[0]


if __name__ == "__main__":
    build_program()
    print("build ok")

